# revision 1
# baseline (speedup 1.0000x reference)
"""MoE feed-forward (top-2 of 8 experts) on 8 Trainium2 NeuronCores.

Strategy (expert-parallel, per the sharding hint):
  - The router (logits -> top-2 -> softmax gates) is the shard-assignment
    computation: it decides which tokens go to which core. It is 0.05% of
    the FLOPs and runs on the host as part of input sharding/dispatch.
  - Core e holds expert e's weights (E == n_cores == 8) and runs the
    dense MLP  gelu(x_e @ W1[e]) @ W2[e]  over the tokens routed to it,
    padded to a common capacity C so all cores run one SPMD program.
  - Everything on device is laid out transposed (tokens in the matmul
    free dim) so no on-device transposes are needed:
        hT[f, t] = sum_d W1[d, f] * xT[d, t]      (lhsT = W1 as stored)
        yT[d, t] = sum_f W2[f, d] * gelu(hT[f, t]) (lhsT = W2 as stored)
  - Matmuls run in bf16 (4x faster than fp32 on the PE) with fp32 PSUM
    accumulation; gelu in fp32 on the scalar engine; output fp32.
  - Host combine: out[tok] += gate * yT.T (scatter-add; each token
    appears at most once per expert so fancy-index add is exact).
"""

import numpy as np
import ml_dtypes

D = 1024
F = 4096
E = 8
TOPK = 2
P = 128

_BASS_CACHE: dict = {}


def _build_bass(
    C: int,
    d: int = D,
    f: int = F,
    tok_tile: int = 256,
    act: str = "Gelu",
    repeat: int = 1,
):
    import concourse.mybir as mybir
    import concourse.tile as tile
    from concourse import bacc

    act_fn = getattr(mybir.ActivationFunctionType, act)

    bf16 = mybir.dt.bfloat16
    f32 = mybir.dt.float32

    # Bacc (not plain Bass): its compile pipeline runs
    # generate_event_semaphores, which splits multi-sem waits into
    # InstEventSemaphore preludes — TRN2 instructions encode only 1 wait.
    nc = bacc.Bacc("TRN2", target_bir_lowering=False, debug=False, num_devices=E)
    xT = nc.declare_dram_parameter("xT", [d, C], bf16, isOutput=False)
    w1 = nc.declare_dram_parameter("w1", [d, f], bf16, isOutput=False)
    w2 = nc.declare_dram_parameter("w2", [f, d], bf16, isOutput=False)
    yT = nc.declare_dram_parameter("yT", [d, C], f32, isOutput=True)

    KD = d // P  # contraction tiles for mm1 / output d-tiles for mm2
    KF = f // P  # f-tiles for mm1 output / contraction tiles for mm2

    tts = []
    off = 0
    while off < C:
        tw = min(tok_tile, C - off)
        tts.append((off, tw))
        off += tw

    # HW-DGE DMA instructions can encode only ONE semaphore wait, and
    # once ~48 HW-DGE DMAs are in flight Tile adds a descriptor-ring
    # recycle wait to every later DMA. Stores inherently need a
    # data-ready wait, so the whole kernel must stay under ~48 DMAs:
    # batch the weight loads into one DMA each, one DMA per xt tile
    # (single-use slots, so no WAR/WAW waits), one store per token tile.
    with tile.TileContext(nc) as tc:
        with (
            tc.tile_pool(name="wpool", bufs=1) as wpool,
            tc.tile_pool(name="xpool", bufs=len(tts)) as xpool,
            tc.tile_pool(name="hpool", bufs=KF + 1) as hpool,
            tc.tile_pool(name="ypool", bufs=1) as ypool,
            tc.tile_pool(name="psum", bufs=8, space="PSUM") as psum_pool,
        ):
            w1_sb = wpool.tile([P, KD, f], bf16)
            nc.scalar.dma_start(w1_sb[:], w1.ap().rearrange("(ko p) f -> p ko f", p=P))
            w2_sb = wpool.tile([P, KF, d], bf16)
            nc.scalar.dma_start(w2_sb[:], w2.ap().rearrange("(ko p) f -> p ko f", p=P))

            xT_t = xT.ap().rearrange("(ko p) c -> p ko c", p=P)
            yT_t = yT.ap().rearrange("(ko p) c -> p ko c", p=P)
            xt_tiles = []
            for t0, tw in tts:
                xt = xpool.tile([P, KD, tok_tile], bf16, tag="xt", name="xt")[:, :, :tw]
                nc.scalar.dma_start(xt[:], xT_t[:, :, t0 : t0 + tw])
                xt_tiles.append(xt)

            for (t0, tw), xt in list(zip(tts, xt_tiles)) * repeat:
                h_tiles = []
                for ft in range(KF):
                    ps = psum_pool.tile([P, tok_tile], f32, tag="ps", name="ps")[:, :tw]
                    for k in range(KD):
                        nc.tensor.matmul(
                            ps[:],
                            w1_sb[:, k, ft * P : (ft + 1) * P],
                            xt[:, k, :],
                            start=(k == 0),
                            stop=(k == KD - 1),
                        )
                    h = hpool.tile([P, tok_tile], bf16, tag="h", name="h")[:, :tw]
                    nc.scalar.activation(h[:], ps[:], act_fn)
                    h_tiles.append(h)

                yt = ypool.tile([P, KD, tok_tile], f32, tag="yt", name="yt")[:, :, :tw]
                # wait-absorber: this DVE write takes on the slot's WAR
                # (previous store's DMA lane); the DVE copies below then
                # depend only on {PE, DVE} and the store only on {DVE} —
                # HW instruction encodings have very few sem-wait slots
                nc.vector.memset(yt[:], 0.0)
                for dt_ in range(KD):
                    ps2 = psum_pool.tile([P, tok_tile], f32, tag="ps", name="ps")[
                        :, :tw
                    ]
                    for ft in range(KF):
                        nc.tensor.matmul(
                            ps2[:],
                            w2_sb[:, ft, dt_ * P : (dt_ + 1) * P],
                            h_tiles[ft][:],
                            start=(ft == 0),
                            stop=(ft == KF - 1),
                        )
                    nc.vector.tensor_copy(yt[:, dt_, :], ps2[:])
                nc.sync.dma_start(yT_t[:, :, t0 : t0 + tw], yt[:])

    nc.compile()  # Bacc pipeline: reg alloc + wait splitting (1 wait/inst on TRN2)
    return nc


def _build_bass_v2(
    C: int,
    d: int = D,
    f: int = F,
    tok_tile: int = 512,
    act: str = "Gelu",
    repeat: int = 1,
):
    """v2: full-PSUM-bank matmuls (N=512) amortize per-MM issue/LDWEIGHTS
    overhead 2x vs 256; y staged and stored as bf16 (halves DVE copy and
    store-DMA traffic; adds ~1e-3 rel err, far under the 2e-2 budget)."""
    import concourse.mybir as mybir
    import concourse.tile as tile
    from concourse import bacc

    act_fn = getattr(mybir.ActivationFunctionType, act)

    bf16 = mybir.dt.bfloat16
    f32 = mybir.dt.float32

    nc = bacc.Bacc("TRN2", target_bir_lowering=False, debug=False, num_devices=E)
    xT = nc.declare_dram_parameter("xT", [d, C], bf16, isOutput=False)
    w1 = nc.declare_dram_parameter("w1", [d, f], bf16, isOutput=False)
    w2 = nc.declare_dram_parameter("w2", [f, d], bf16, isOutput=False)
    yT = nc.declare_dram_parameter("yT", [d, C], bf16, isOutput=True)

    KD = d // P
    KF = f // P

    tts = []
    off = 0
    while off < C:
        tw = min(tok_tile, C - off)
        tts.append((off, tw))
        off += tw

    # hpool NEEDS >= KF bufs: mm2's last d-group reads every h tile, so all
    # KF h tiles of one iteration are live at once (KF-1 deadlocks).
    # xt tiles are sized per width (tag per tw) so the 128-wide tail doesn't
    # pay for a full 512-wide slot — SBUF is within ~4KB of full here.
    from collections import Counter

    tw_counts = Counter(tw for _, tw in tts)
    with tile.TileContext(nc) as tc:
        with (
            tc.tile_pool(name="wpool", bufs=1) as wpool,
            tc.tile_pool(name="xpool", bufs=1) as xpool,
            tc.tile_pool(name="hpool", bufs=KF) as hpool,
            tc.tile_pool(name="ypool", bufs=1) as ypool,
            tc.tile_pool(name="psum", bufs=8, space="PSUM") as psum_pool,
        ):
            w1_sb = wpool.tile([P, KD, f], bf16)
            nc.scalar.dma_start(w1_sb[:], w1.ap().rearrange("(ko p) f -> p ko f", p=P))
            w2_sb = wpool.tile([P, KF, d], bf16)
            nc.scalar.dma_start(w2_sb[:], w2.ap().rearrange("(ko p) f -> p ko f", p=P))

            xT_t = xT.ap().rearrange("(ko p) c -> p ko c", p=P)
            yT_t = yT.ap().rearrange("(ko p) c -> p ko c", p=P)
            xt_tiles = []
            for t0, tw in tts:
                xt = xpool.tile(
                    [P, KD, tw], bf16, tag=f"xt{tw}", bufs=tw_counts[tw], name="xt"
                )
                nc.scalar.dma_start(xt[:], xT_t[:, :, t0 : t0 + tw])
                xt_tiles.append(xt)

            for (t0, tw), xt in list(zip(tts, xt_tiles)) * repeat:
                h_tiles = []
                for ft in range(KF):
                    ps = psum_pool.tile([P, tok_tile], f32, tag="ps", name="ps")[:, :tw]
                    for k in range(KD):
                        nc.tensor.matmul(
                            ps[:],
                            w1_sb[:, k, ft * P : (ft + 1) * P],
                            xt[:, k, :],
                            start=(k == 0),
                            stop=(k == KD - 1),
                        )
                    h = hpool.tile([P, tok_tile], bf16, tag="h", name="h")[:, :tw]
                    nc.scalar.activation(h[:], ps[:], act_fn)
                    h_tiles.append(h)

                yt = ypool.tile([P, KD, tok_tile], bf16, tag="yt", name="yt")[:, :, :tw]
                nc.vector.memset(yt[:], 0.0)
                for dt_ in range(KD):
                    ps2 = psum_pool.tile([P, tok_tile], f32, tag="ps", name="ps")[
                        :, :tw
                    ]
                    for ft in range(KF):
                        nc.tensor.matmul(
                            ps2[:],
                            w2_sb[:, ft, dt_ * P : (dt_ + 1) * P],
                            h_tiles[ft][:],
                            start=(ft == 0),
                            stop=(ft == KF - 1),
                        )
                    nc.vector.tensor_copy(yt[:, dt_, :], ps2[:])
                nc.sync.dma_start(yT_t[:, :, t0 : t0 + tw], yt[:])

    nc.compile()
    return nc


def _build_bass_v3(
    counts,
    d: int = D,
    f_loc: int = F // E,
    tok_tile: int = 512,
    act: str = "Gelu",
    repeat: int = 1,
):
    """v3: 8-way F-sharding, zero capacity padding.

    Every core processes ALL Ntot = sum(counts) token-expert columns
    (columns sorted by expert, ragged per-expert tails — no padding),
    holding an F/8 = 512 slice of every expert's W1/W2. Per-column PE
    work drops 8x vs a full expert, but there are 8x the columns, so
    total cycles = Ntot*(2*D*f_loc/128) = 1.05M vs v2's C*512 = 1.11M:
    the 6% capacity-padding tax (C = max expert count, padded to 128)
    is gone entirely. Outputs are F-slice partials; the host sums the 8
    cores' partials (bf16 partials add ~2e-3 rel err, budget is 2e-2).

    xT no longer fits in SBUF (33 MB) and is double-buffer streamed per
    tile; loads ride the scalar-engine DGE ring, stores the sync ring,
    so neither queue's FIFO blocks the other.
    """
    import concourse.mybir as mybir
    import concourse.tile as tile
    from concourse import bacc

    act_fn = getattr(mybir.ActivationFunctionType, act)

    bf16 = mybir.dt.bfloat16
    f32 = mybir.dt.float32

    Ntot = int(sum(counts))
    f_all = f_loc * len(counts)  # per-core weight columns, expert-major

    nc = bacc.Bacc("TRN2", target_bir_lowering=False, debug=False, num_devices=E)
    xT = nc.declare_dram_parameter("xT", [d, Ntot], bf16, isOutput=False)
    w1 = nc.declare_dram_parameter("w1", [d, f_all], bf16, isOutput=False)
    w2 = nc.declare_dram_parameter("w2", [f_all, d], bf16, isOutput=False)
    yT = nc.declare_dram_parameter("yT", [d, Ntot], bf16, isOutput=True)

    KD = d // P  # contraction tiles for mm1 / output d-tiles for mm2
    KFL = f_loc // P  # f-tiles per expert slice (4)

    # (expert, col_offset, width) tile schedule: ragged per-expert tails.
    tiles_sched = []
    off = 0
    for e, cnt in enumerate(counts):
        seg = 0
        while seg < cnt:
            tw = min(tok_tile, cnt - seg)
            tiles_sched.append((e, off + seg, tw))
            seg += tw
        off += cnt

    with tile.TileContext(nc) as tc:
        with (
            tc.tile_pool(name="wpool", bufs=1) as wpool,
            tc.tile_pool(name="xpool", bufs=3) as xpool,
            tc.tile_pool(name="hpool", bufs=KFL + 1) as hpool,
            tc.tile_pool(name="ypool", bufs=2) as ypool,
            tc.tile_pool(name="psum", bufs=8, space="PSUM") as psum_pool,
        ):
            w1_sb = wpool.tile([P, KD, f_all], bf16)
            nc.scalar.dma_start(w1_sb[:], w1.ap().rearrange("(ko p) f -> p ko f", p=P))
            w2_sb = wpool.tile([P, KFL * len(counts), d], bf16)
            nc.scalar.dma_start(w2_sb[:], w2.ap().rearrange("(ko p) f -> p ko f", p=P))

            xT_t = xT.ap().rearrange("(ko p) c -> p ko c", p=P)
            yT_t = yT.ap().rearrange("(ko p) c -> p ko c", p=P)

            # xt loads are emitted two tiles ahead of their consumer (the
            # ACT queue is FIFO, so emission position sets when the DMA
            # starts): a load issued at tile t's start has ~1.5 tiles of
            # PE work to cover its ~3us transfer. bufs=3 ring matches.
            sched = tiles_sched * repeat
            xts: dict = {}

            def emit_load(i):
                _, lt0, ltw = sched[i]
                xt = xpool.tile([P, KD, tok_tile], bf16, tag="xt", name="xt")[
                    :, :, :ltw
                ]
                nc.scalar.dma_start(xt[:], xT_t[:, :, lt0 : lt0 + ltw])
                xts[i] = xt

            emit_load(0)
            if len(sched) > 1:
                emit_load(1)

            for i, (e, t0, tw) in enumerate(sched):
                if i + 2 < len(sched):
                    emit_load(i + 2)
                xt = xts.pop(i)

                h_tiles = []
                for ft in range(KFL):
                    fcol = (e * KFL + ft) * P
                    ps = psum_pool.tile([P, tok_tile], f32, tag="ps", name="ps")[:, :tw]
                    for k in range(KD):
                        nc.tensor.matmul(
                            ps[:],
                            w1_sb[:, k, fcol : fcol + P],
                            xt[:, k, :],
                            start=(k == 0),
                            stop=(k == KD - 1),
                        )
                    h = hpool.tile([P, tok_tile], bf16, tag="h", name="h")[:, :tw]
                    nc.scalar.activation(h[:], ps[:], act_fn)
                    h_tiles.append(h)

                # bufs=2 on ypool: the WAR on this slot is the store DMA
                # from two tiles back (~25us of PE work ago) — no absorber
                # memset needed, the first DVE copy's split wait is free.
                yt = ypool.tile([P, KD, tok_tile], bf16, tag="yt", name="yt")[
                    :, :, :tw
                ]
                for dt_ in range(KD):
                    ps2 = psum_pool.tile([P, tok_tile], f32, tag="ps", name="ps")[
                        :, :tw
                    ]
                    for ft in range(KFL):
                        nc.tensor.matmul(
                            ps2[:],
                            w2_sb[:, e * KFL + ft, dt_ * P : (dt_ + 1) * P],
                            h_tiles[ft][:],
                            start=(ft == 0),
                            stop=(ft == KFL - 1),
                        )
                    nc.vector.tensor_copy(yt[:, dt_, :], ps2[:])
                nc.sync.dma_start(yT_t[:, :, t0 : t0 + tw], yt[:])

    nc.compile()
    return nc


def _route(xf: np.ndarray, Wr: np.ndarray):
    """Top-2 routing on the host (fp64 logits for a stable ranking)."""
    logits = xf.astype(np.float64) @ Wr.astype(np.float64).T  # [N, E]
    order = np.argsort(-logits, axis=1)[:, :TOPK]  # [N, 2] expert ids, desc
    top_vals = np.take_along_axis(logits, order, axis=1).astype(np.float32)
    m = top_vals.max(axis=1, keepdims=True)
    ex = np.exp(top_vals - m)
    gates2 = (ex / ex.sum(axis=1, keepdims=True)).astype(np.float32)  # [N, 2]
    return order, gates2


def _dispatch_v3(inputs):
    """Host routing + v3 input construction.

    Returns (in_maps, counts, combine) where combine(yT_sum_fp32) -> out.
    Columns are token-expert pairs sorted by expert; every core receives
    the same xT and its own F/8 slice of all experts' weights.
    """
    x = np.asarray(inputs["x"], dtype=np.float32)
    Wr = np.asarray(inputs["Wr"], dtype=np.float32)
    W1 = np.asarray(inputs["W1"], dtype=np.float32)
    W2 = np.asarray(inputs["W2"], dtype=np.float32)
    B, T, d = x.shape
    N = B * T
    f = W1.shape[2]
    f_loc = f // E
    xf = np.ascontiguousarray(x.reshape(N, d))

    order, gates2 = _route(xf, Wr)
    idx_list, gate_list = [], []
    for e in range(E):
        tok, slot = np.where(order == e)
        idx_list.append(tok)
        gate_list.append(gates2[tok, slot])
    counts = tuple(len(t) for t in idx_list)

    perm_tokens = np.concatenate(idx_list)
    xT = np.ascontiguousarray(xf.astype(ml_dtypes.bfloat16)[perm_tokens].T)

    in_maps = []
    for c in range(E):
        w1_c = np.concatenate(
            [W1[e][:, c * f_loc : (c + 1) * f_loc] for e in range(E)], axis=1
        )
        w2_c = np.concatenate(
            [W2[e][c * f_loc : (c + 1) * f_loc, :] for e in range(E)], axis=0
        )
        in_maps.append(
            {
                "xT": xT,
                "w1": np.ascontiguousarray(w1_c).astype(ml_dtypes.bfloat16),
                "w2": np.ascontiguousarray(w2_c).astype(ml_dtypes.bfloat16),
            }
        )

    def combine(y_sum: np.ndarray) -> np.ndarray:
        # y_sum: [d, Ntot] fp32 (partials already summed over cores)
        out = np.zeros((N, d), dtype=np.float32)
        off = 0
        for e in range(E):
            tok, gate = idx_list[e], gate_list[e]
            out[tok] += gate[:, None] * y_sum[:, off : off + len(tok)].T
            off += len(tok)
        return out.reshape(B, T, d)

    return in_maps, counts, combine


def _run(inputs, trace: bool = False, variant: str = "v4"):
    if variant == "v4":
        in_maps, counts, combine = _dispatch_v4(inputs)
        builder = _build_bass_v4
    else:
        in_maps, counts, combine = _dispatch_v3(inputs)
        builder = _build_bass_v3

    key = (variant, counts)
    if key not in _BASS_CACHE:
        _BASS_CACHE[key] = builder(counts)
    nc = _BASS_CACHE[key]

    from concourse.bass_utils import run_bass_kernel_spmd

    res = run_bass_kernel_spmd(nc, in_maps, core_ids=list(range(E)), trace=trace)

    y_sum = np.zeros(res.results[0]["yT"].shape, dtype=np.float32)
    for c in range(E):
        y_sum += np.asarray(res.results[c]["yT"]).astype(np.float32)
    return combine(y_sum), res


def kernel(**inputs) -> np.ndarray:
    out, _ = _run(inputs, trace=False)
    return out


def _build_floor(n_groups: int = 272, gsz: int = 8, tok: int = 512, repeat: int = 1):
    """PE-roofline probe: back-to-back bf16 matmul groups (N=tok), no
    ACT/DVE/DMA consumers — measures the achievable streaming floor of
    the current chip clock epoch for slope comparison against kernels."""
    import concourse.mybir as mybir
    import concourse.tile as tile
    from concourse import bacc

    bf16 = mybir.dt.bfloat16
    f32 = mybir.dt.float32

    nc = bacc.Bacc("TRN2", target_bir_lowering=False, debug=False, num_devices=E)
    w1 = nc.declare_dram_parameter("w1", [D, F], bf16, isOutput=False)
    yT = nc.declare_dram_parameter("yT", [P, tok], f32, isOutput=True)

    with tile.TileContext(nc) as tc:
        with (
            tc.tile_pool(name="wpool", bufs=1) as wpool,
            tc.tile_pool(name="psum", bufs=8, space="PSUM") as psum_pool,
        ):
            w_sb = wpool.tile([P, 8, F], bf16)
            nc.scalar.dma_start(w_sb[:], w1.ap().rearrange("(ko p) f -> p ko f", p=P))
            xt = wpool.tile([P, 8, tok], bf16)
            nc.vector.memset(xt[:], 0.25)

            ps = None
            for g in range(n_groups * repeat):
                ps = psum_pool.tile([P, tok], f32, tag="ps", name="ps")
                for k in range(gsz):
                    nc.tensor.matmul(
                        ps[:],
                        w_sb[:, k, (g % 32) * P : (g % 32) * P + P],
                        xt[:, k, :],
                        start=(k == 0),
                        stop=(k == gsz - 1),
                    )
            out = wpool.tile([P, tok], f32)
            nc.vector.tensor_copy(out[:], ps[:])
            nc.sync.dma_start(yT.ap(), out[:])

    nc.compile()
    return nc


def _build_floor_act(
    n_groups: int = 272,
    gsz: int = 8,
    tok: int = 512,
    repeat: int = 1,
    drain: str = "act",
):
    """Floor probe + per-group PSUM drain (ACT gelu or DVE copy) to SBUF,
    mimicking v2's mm1 consumer pattern — isolates the cost of engines
    draining PSUM while the PE streams."""
    import concourse.mybir as mybir
    import concourse.tile as tile
    from concourse import bacc

    bf16 = mybir.dt.bfloat16
    f32 = mybir.dt.float32
    act_fn = mybir.ActivationFunctionType.Gelu

    nc = bacc.Bacc("TRN2", target_bir_lowering=False, debug=False, num_devices=E)
    w1 = nc.declare_dram_parameter("w1", [D, F], bf16, isOutput=False)
    yT = nc.declare_dram_parameter("yT", [P, tok], f32, isOutput=True)

    with tile.TileContext(nc) as tc:
        with (
            tc.tile_pool(name="wpool", bufs=1) as wpool,
            tc.tile_pool(name="hpool", bufs=8) as hpool,
            tc.tile_pool(name="psum", bufs=8, space="PSUM") as psum_pool,
        ):
            w_sb = wpool.tile([P, 8, F], bf16)
            nc.scalar.dma_start(w_sb[:], w1.ap().rearrange("(ko p) f -> p ko f", p=P))
            xt = wpool.tile([P, 8, tok], bf16)
            nc.vector.memset(xt[:], 0.25)

            h = None
            for g in range(n_groups * repeat):
                ps = psum_pool.tile([P, tok], f32, tag="ps", name="ps")
                for k in range(gsz):
                    nc.tensor.matmul(
                        ps[:],
                        w_sb[:, k, (g % 32) * P : (g % 32) * P + P],
                        xt[:, k, :],
                        start=(k == 0),
                        stop=(k == gsz - 1),
                    )
                h = hpool.tile([P, tok], bf16, tag="h", name="h")
                if drain == "act":
                    nc.scalar.activation(h[:], ps[:], act_fn)
                else:
                    nc.vector.tensor_copy(h[:], ps[:])
            out = wpool.tile([P, tok], f32)
            nc.vector.tensor_copy(out[:], h[:])
            nc.sync.dma_start(yT.ap(), out[:])

    nc.compile()
    return nc


def _build_v2mm(C: int = 2176, tok_tile: int = 512, repeat: int = 1):
    """v2's exact MM stream (mm1/mm2 group alternation, tail tile) with
    no ACT/DVE/store consumers — isolates MM-stream structure cost."""
    import concourse.mybir as mybir
    import concourse.tile as tile
    from concourse import bacc

    bf16 = mybir.dt.bfloat16
    f32 = mybir.dt.float32

    nc = bacc.Bacc("TRN2", target_bir_lowering=False, debug=False, num_devices=E)
    w1 = nc.declare_dram_parameter("w1", [D, F], bf16, isOutput=False)
    w2 = nc.declare_dram_parameter("w2", [F, D], bf16, isOutput=False)
    yT = nc.declare_dram_parameter("yT", [P, tok_tile], f32, isOutput=True)

    KD, KF = D // P, F // P
    tts = []
    off = 0
    while off < C:
        tw = min(tok_tile, C - off)
        tts.append((off, tw))
        off += tw

    with tile.TileContext(nc) as tc:
        with (
            tc.tile_pool(name="wpool", bufs=1) as wpool,
            tc.tile_pool(name="psum", bufs=8, space="PSUM") as psum_pool,
        ):
            w1_sb = wpool.tile([P, KD, F], bf16)
            nc.scalar.dma_start(w1_sb[:], w1.ap().rearrange("(ko p) f -> p ko f", p=P))
            w2_sb = wpool.tile([P, KF, D], bf16)
            nc.scalar.dma_start(w2_sb[:], w2.ap().rearrange("(ko p) f -> p ko f", p=P))
            xt = wpool.tile([P, KD, tok_tile], bf16)
            nc.vector.memset(xt[:], 0.25)
            h_tiles = [wpool.tile([P, tok_tile], bf16, name=f"h{i}") for i in range(KF)]
            for h in h_tiles:
                nc.vector.memset(h[:], 0.25)

            ps = None
            for t0, tw in tts * repeat:
                for ft in range(KF):
                    ps = psum_pool.tile([P, tok_tile], f32, tag="ps", name="ps")[:, :tw]
                    for k in range(KD):
                        nc.tensor.matmul(
                            ps[:],
                            w1_sb[:, k, ft * P : (ft + 1) * P],
                            xt[:, k, :tw],
                            start=(k == 0),
                            stop=(k == KD - 1),
                        )
                for dt_ in range(KD):
                    ps2 = psum_pool.tile([P, tok_tile], f32, tag="ps", name="ps")[:, :tw]
                    for ft in range(KF):
                        nc.tensor.matmul(
                            ps2[:],
                            w2_sb[:, ft, dt_ * P : (dt_ + 1) * P],
                            h_tiles[ft][:, :tw],
                            start=(ft == 0),
                            stop=(ft == KF - 1),
                        )
            out = wpool.tile([P, tok_tile], f32)
            last_tw = tts[-1][1]
            nc.vector.tensor_copy(out[:, :last_tw], ps[:])
            nc.sync.dma_start(yT.ap(), out[:])

    nc.compile()
    return nc


def _build_v2probe(
    C: int = 2176,
    tok_tile: int = 512,
    repeat: int = 1,
    with_act: bool = True,
    with_out: bool = True,
):
    """v2 with selectively removed consumer stages, for overhead bisection:
    with_act=False -> mm2 reads pre-set h (no ACT deps);
    with_out=False -> no yt memset/DVE copies/stores (mm2 psum undrained)."""
    import concourse.mybir as mybir
    import concourse.tile as tile
    from concourse import bacc

    bf16 = mybir.dt.bfloat16
    f32 = mybir.dt.float32
    act_fn = mybir.ActivationFunctionType.Gelu

    nc = bacc.Bacc("TRN2", target_bir_lowering=False, debug=False, num_devices=E)
    xT = nc.declare_dram_parameter("xT", [D, C], bf16, isOutput=False)
    w1 = nc.declare_dram_parameter("w1", [D, F], bf16, isOutput=False)
    w2 = nc.declare_dram_parameter("w2", [F, D], bf16, isOutput=False)
    yT = nc.declare_dram_parameter("yT", [D, C], bf16, isOutput=True)

    KD, KF = D // P, F // P
    tts = []
    off = 0
    while off < C:
        tw = min(tok_tile, C - off)
        tts.append((off, tw))
        off += tw
    from collections import Counter

    tw_counts = Counter(tw for _, tw in tts)
    with tile.TileContext(nc) as tc:
        with (
            tc.tile_pool(name="wpool", bufs=1) as wpool,
            tc.tile_pool(name="xpool", bufs=1) as xpool,
            tc.tile_pool(name="hpool", bufs=KF) as hpool,
            tc.tile_pool(name="ypool", bufs=1) as ypool,
            tc.tile_pool(name="psum", bufs=8, space="PSUM") as psum_pool,
        ):
            w1_sb = wpool.tile([P, KD, F], bf16)
            nc.scalar.dma_start(w1_sb[:], w1.ap().rearrange("(ko p) f -> p ko f", p=P))
            w2_sb = wpool.tile([P, KF, D], bf16)
            nc.scalar.dma_start(w2_sb[:], w2.ap().rearrange("(ko p) f -> p ko f", p=P))

            xT_t = xT.ap().rearrange("(ko p) c -> p ko c", p=P)
            yT_t = yT.ap().rearrange("(ko p) c -> p ko c", p=P)
            xt_tiles = []
            for t0, tw in tts:
                xt = xpool.tile(
                    [P, KD, tw], bf16, tag=f"xt{tw}", bufs=tw_counts[tw], name="xt"
                )
                nc.scalar.dma_start(xt[:], xT_t[:, :, t0 : t0 + tw])
                xt_tiles.append(xt)

            h_fixed = None
            if not with_act:
                h_fixed = [
                    wpool.tile([P, tok_tile], bf16, name=f"hf{i}") for i in range(KF)
                ]
                for h in h_fixed:
                    nc.vector.memset(h[:], 0.25)

            for (t0, tw), xt in list(zip(tts, xt_tiles)) * repeat:
                h_tiles = []
                for ft in range(KF):
                    ps = psum_pool.tile([P, tok_tile], f32, tag="ps", name="ps")[:, :tw]
                    for k in range(KD):
                        nc.tensor.matmul(
                            ps[:],
                            w1_sb[:, k, ft * P : (ft + 1) * P],
                            xt[:, k, :],
                            start=(k == 0),
                            stop=(k == KD - 1),
                        )
                    if with_act:
                        h = hpool.tile([P, tok_tile], bf16, tag="h", name="h")[:, :tw]
                        nc.scalar.activation(h[:], ps[:], act_fn)
                        h_tiles.append(h)
                    else:
                        h_tiles.append(h_fixed[ft][:, :tw])

                if with_out:
                    yt = ypool.tile([P, KD, tok_tile], bf16, tag="yt", name="yt")[
                        :, :, :tw
                    ]
                    nc.vector.memset(yt[:], 0.0)
                for dt_ in range(KD):
                    ps2 = psum_pool.tile([P, tok_tile], f32, tag="ps", name="ps")[
                        :, :tw
                    ]
                    for ft in range(KF):
                        nc.tensor.matmul(
                            ps2[:],
                            w2_sb[:, ft, dt_ * P : (dt_ + 1) * P],
                            h_tiles[ft][:],
                            start=(ft == 0),
                            stop=(ft == KF - 1),
                        )
                    if with_out:
                        nc.vector.tensor_copy(yt[:, dt_, :], ps2[:])
                if with_out:
                    nc.sync.dma_start(yT_t[:, :, t0 : t0 + tw], yt[:])

    nc.compile()
    return nc


def _build_v3probe(
    counts=None,
    tok_tile: int = 512,
    repeat: int = 1,
    with_loads: bool = True,
    with_act: bool = True,
    with_out: bool = True,
):
    """v3 with selectively removed stages (timing probe, wrong results):
    with_loads=False -> one resident memset xt reused for every tile;
    with_act=False -> mm2 reads pre-set h tiles; with_out=False -> no
    yt copies/stores."""
    import concourse.mybir as mybir
    import concourse.tile as tile
    from concourse import bacc

    if counts is None:
        counts = (2043, 1968, 2056, 2175, 2017, 2028, 2050, 2047)
    bf16 = mybir.dt.bfloat16
    f32 = mybir.dt.float32
    act_fn = mybir.ActivationFunctionType.Gelu

    f_loc = F // E
    Ntot = int(sum(counts))
    f_all = f_loc * len(counts)

    nc = bacc.Bacc("TRN2", target_bir_lowering=False, debug=False, num_devices=E)
    xT = nc.declare_dram_parameter("xT", [D, Ntot], bf16, isOutput=False)
    w1 = nc.declare_dram_parameter("w1", [D, f_all], bf16, isOutput=False)
    w2 = nc.declare_dram_parameter("w2", [f_all, D], bf16, isOutput=False)
    yT = nc.declare_dram_parameter("yT", [D, Ntot], bf16, isOutput=True)

    KD = D // P
    KFL = f_loc // P

    tiles_sched = []
    off = 0
    for e, cnt in enumerate(counts):
        seg = 0
        while seg < cnt:
            tw = min(tok_tile, cnt - seg)
            tiles_sched.append((e, off + seg, tw))
            seg += tw
        off += cnt

    with tile.TileContext(nc) as tc:
        with (
            tc.tile_pool(name="wpool", bufs=1) as wpool,
            tc.tile_pool(name="xpool", bufs=3) as xpool,
            tc.tile_pool(name="hpool", bufs=KFL + 1) as hpool,
            tc.tile_pool(name="ypool", bufs=2) as ypool,
            tc.tile_pool(name="psum", bufs=8, space="PSUM") as psum_pool,
        ):
            w1_sb = wpool.tile([P, KD, f_all], bf16)
            nc.scalar.dma_start(w1_sb[:], w1.ap().rearrange("(ko p) f -> p ko f", p=P))
            w2_sb = wpool.tile([P, KFL * len(counts), D], bf16)
            nc.scalar.dma_start(w2_sb[:], w2.ap().rearrange("(ko p) f -> p ko f", p=P))

            xT_t = xT.ap().rearrange("(ko p) c -> p ko c", p=P)
            yT_t = yT.ap().rearrange("(ko p) c -> p ko c", p=P)

            xt_fixed = None
            if not with_loads:
                xt_fixed = wpool.tile([P, KD, tok_tile], bf16)
                nc.vector.memset(xt_fixed[:], 0.25)
            h_fixed = None
            if not with_act:
                h_fixed = [
                    wpool.tile([P, tok_tile], bf16, name=f"hf{i}") for i in range(KFL)
                ]
                for h in h_fixed:
                    nc.vector.memset(h[:], 0.25)

            sched = tiles_sched * repeat
            xts: dict = {}

            def emit_load(i):
                _, lt0, ltw = sched[i]
                xt = xpool.tile([P, KD, tok_tile], bf16, tag="xt", name="xt")[
                    :, :, :ltw
                ]
                nc.scalar.dma_start(xt[:], xT_t[:, :, lt0 : lt0 + ltw])
                xts[i] = xt

            if with_loads:
                emit_load(0)
                if len(sched) > 1:
                    emit_load(1)

            for i, (e, t0, tw) in enumerate(sched):
                if with_loads:
                    if i + 2 < len(sched):
                        emit_load(i + 2)
                    xt = xts.pop(i)
                else:
                    xt = xt_fixed[:, :, :tw]

                h_tiles = []
                for ft in range(KFL):
                    fcol = (e * KFL + ft) * P
                    ps = psum_pool.tile([P, tok_tile], f32, tag="ps", name="ps")[:, :tw]
                    for k in range(KD):
                        nc.tensor.matmul(
                            ps[:],
                            w1_sb[:, k, fcol : fcol + P],
                            xt[:, k, :],
                            start=(k == 0),
                            stop=(k == KD - 1),
                        )
                    if with_act:
                        h = hpool.tile([P, tok_tile], bf16, tag="h", name="h")[:, :tw]
                        nc.scalar.activation(h[:], ps[:], act_fn)
                        h_tiles.append(h)
                    else:
                        h_tiles.append(h_fixed[ft][:, :tw])

                if with_out:
                    yt = ypool.tile([P, KD, tok_tile], bf16, tag="yt", name="yt")[
                        :, :, :tw
                    ]
                for dt_ in range(KD):
                    ps2 = psum_pool.tile([P, tok_tile], f32, tag="ps", name="ps")[
                        :, :tw
                    ]
                    for ft in range(KFL):
                        nc.tensor.matmul(
                            ps2[:],
                            w2_sb[:, e * KFL + ft, dt_ * P : (dt_ + 1) * P],
                            h_tiles[ft][:],
                            start=(ft == 0),
                            stop=(ft == KFL - 1),
                        )
                    if with_out:
                        nc.vector.tensor_copy(yt[:, dt_, :], ps2[:])
                if with_out:
                    nc.sync.dma_start(yT_t[:, :, t0 : t0 + tw], yt[:])

    nc.compile()
    return nc


def _v4_sched(counts, tok_tile: int = 512):
    """(expert, col_offset, width, slab_offset) tile schedule; slabs are
    tile-contiguous regions of KD*tw columns in the device xT/yT layout.
    Each expert's columns split into near-equal widths (<= tok_tile)
    rather than 512-chunks + remainder: a 2-column tail tile still costs
    64 matmuls at the ~60-cycle instruction floor, balanced widths don't."""
    KD = D // P
    sched = []
    off = 0
    slab = 0
    for e, cnt in enumerate(counts):
        n_t = max(1, -(-cnt // tok_tile))
        seg = 0
        for j in range(n_t):
            tw = -(-(cnt - seg) // (n_t - j))  # ceil split of the remainder
            sched.append((e, off + seg, tw, slab))
            slab += KD * tw
            seg += tw
        off += cnt
    return sched, slab


def _build_bass_v4(
    counts,
    tok_tile: int = 512,
    act: str = "Gelu",
    repeat: int = 1,
):
    """v4 = v3 (8-way F-sharding, zero padding) + tile-contiguous HBM
    layout for xT/yT: each per-tile DMA moves one [P, KD*tw] slab that is
    contiguous per partition (8KB descriptors instead of the 1KB scatter
    a [P, KD, Ntot] view produces — v3's DMA cost was ~2x its compute
    saving, all descriptor overhead)."""
    import concourse.mybir as mybir
    import concourse.tile as tile
    from concourse import bacc

    act_fn = getattr(mybir.ActivationFunctionType, act)
    bf16 = mybir.dt.bfloat16
    f32 = mybir.dt.float32

    f_loc = F // E
    f_all = f_loc * len(counts)
    KD = D // P
    KFL = f_loc // P

    sched, totcols = _v4_sched(counts, tok_tile)

    nc = bacc.Bacc("TRN2", target_bir_lowering=False, debug=False, num_devices=E)
    xT = nc.declare_dram_parameter("xT", [P, totcols], bf16, isOutput=False)
    w1 = nc.declare_dram_parameter("w1", [D, f_all], bf16, isOutput=False)
    w2 = nc.declare_dram_parameter("w2", [f_all, D], bf16, isOutput=False)
    yT = nc.declare_dram_parameter("yT", [P, totcols], bf16, isOutput=True)

    LOOKAHEAD = 4  # tiles of xt prefetch depth (ring bufs = LOOKAHEAD + 2)
    with tile.TileContext(nc) as tc:
        with (
            tc.tile_pool(name="wpool", bufs=1) as wpool,
            tc.tile_pool(name="xpool", bufs=LOOKAHEAD + 1) as xpool,
            tc.tile_pool(name="hpool", bufs=KFL + 1) as hpool,
            tc.tile_pool(name="ypool", bufs=3) as ypool,
            tc.tile_pool(name="psum", bufs=8, space="PSUM") as psum_pool,
        ):
            w1_sb = wpool.tile([P, KD, f_all], bf16)
            nc.scalar.dma_start(w1_sb[:], w1.ap().rearrange("(ko p) f -> p ko f", p=P))
            w2_sb = wpool.tile([P, KFL * len(counts), D], bf16)
            nc.scalar.dma_start(w2_sb[:], w2.ap().rearrange("(ko p) f -> p ko f", p=P))

            full_sched = sched * repeat
            xts: dict = {}

            def emit_load(i):
                _, _, ltw, lslab = full_sched[i]
                xt = xpool.tile([P, KD * tok_tile], bf16, tag="xt", name="xt")[
                    :, : KD * ltw
                ]
                nc.scalar.dma_start(xt[:], xT.ap()[:, lslab : lslab + KD * ltw])
                xts[i] = xt

            for j in range(min(LOOKAHEAD, len(full_sched))):
                emit_load(j)

            for i, (e, t0, tw, slab) in enumerate(full_sched):
                if i + LOOKAHEAD < len(full_sched):
                    emit_load(i + LOOKAHEAD)
                xt = xts.pop(i)  # [P, KD*tw], k-th contraction block at k*tw

                h_tiles = []
                for ft in range(KFL):
                    fcol = (e * KFL + ft) * P
                    ps = psum_pool.tile([P, tok_tile], f32, tag="ps", name="ps")[:, :tw]
                    for k in range(KD):
                        nc.tensor.matmul(
                            ps[:],
                            w1_sb[:, k, fcol : fcol + P],
                            xt[:, k * tw : (k + 1) * tw],
                            start=(k == 0),
                            stop=(k == KD - 1),
                        )
                    h = hpool.tile([P, tok_tile], bf16, tag="h", name="h")[:, :tw]
                    nc.scalar.activation(h[:], ps[:], act_fn)
                    h_tiles.append(h)

                yt = ypool.tile([P, KD * tok_tile], bf16, tag="yt", name="yt")[
                    :, : KD * tw
                ]
                for dt_ in range(KD):
                    ps2 = psum_pool.tile([P, tok_tile], f32, tag="ps", name="ps")[
                        :, :tw
                    ]
                    for ft in range(KFL):
                        nc.tensor.matmul(
                            ps2[:],
                            w2_sb[:, e * KFL + ft, dt_ * P : (dt_ + 1) * P],
                            h_tiles[ft][:],
                            start=(ft == 0),
                            stop=(ft == KFL - 1),
                        )
                    nc.vector.tensor_copy(yt[:, dt_ * tw : (dt_ + 1) * tw], ps2[:])
                nc.sync.dma_start(yT.ap()[:, slab : slab + KD * tw], yt[:])

    nc.compile()
    return nc


def _dispatch_v4(inputs, tok_tile: int = 512):
    """Host routing + v4 tile-contiguous input construction."""
    x = np.asarray(inputs["x"], dtype=np.float32)
    Wr = np.asarray(inputs["Wr"], dtype=np.float32)
    W1 = np.asarray(inputs["W1"], dtype=np.float32)
    W2 = np.asarray(inputs["W2"], dtype=np.float32)
    B, T, d = x.shape
    N = B * T
    f = W1.shape[2]
    f_loc = f // E
    KD = d // P
    xf = np.ascontiguousarray(x.reshape(N, d))

    order, gates2 = _route(xf, Wr)
    idx_list, gate_list = [], []
    for e in range(E):
        tok, slot = np.where(order == e)
        idx_list.append(tok)
        gate_list.append(gates2[tok, slot])
    counts = tuple(len(t) for t in idx_list)
    sched, totcols = _v4_sched(counts, tok_tile)

    perm_tokens = np.concatenate(idx_list)
    xT_cols = xf.astype(ml_dtypes.bfloat16)[perm_tokens].T  # [d, Ntot]

    xT_dev = np.empty((P, totcols), dtype=ml_dtypes.bfloat16)
    for e, t0, tw, slab in sched:
        blk = xT_cols[:, t0 : t0 + tw].reshape(KD, P, tw).transpose(1, 0, 2)
        xT_dev[:, slab : slab + KD * tw] = blk.reshape(P, KD * tw)

    in_maps = []
    for c in range(E):
        w1_c = np.concatenate(
            [W1[e][:, c * f_loc : (c + 1) * f_loc] for e in range(E)], axis=1
        )
        w2_c = np.concatenate(
            [W2[e][c * f_loc : (c + 1) * f_loc, :] for e in range(E)], axis=0
        )
        in_maps.append(
            {
                "xT": xT_dev,
                "w1": np.ascontiguousarray(w1_c).astype(ml_dtypes.bfloat16),
                "w2": np.ascontiguousarray(w2_c).astype(ml_dtypes.bfloat16),
            }
        )

    def combine(y_dev_sum: np.ndarray) -> np.ndarray:
        # y_dev_sum: [P, totcols] fp32 (summed over cores); unpack slabs
        y_cols = np.empty((d, sum(counts)), dtype=np.float32)
        for e, t0, tw, slab in sched:
            blk = y_dev_sum[:, slab : slab + KD * tw].reshape(P, KD, tw)
            y_cols[:, t0 : t0 + tw] = blk.transpose(1, 0, 2).reshape(d, tw)
        out = np.zeros((N, d), dtype=np.float32)
        off = 0
        for e in range(E):
            tok, gate = idx_list[e], gate_list[e]
            out[tok] += gate[:, None] * y_cols[:, off : off + len(tok)].T
            off += len(tok)
        return out.reshape(B, T, d)

    return in_maps, counts, combine


def _build_bass_v2c(
    C: int,
    d: int = D,
    f: int = F,
    tok_tile: int = 512,
    act: str = "Gelu",
    repeat: int = 1,
):
    """v2 + tile-contiguous yT store layout: stores write [P, KD*tw] slabs
    (8KB contiguous per partition) instead of 1KB-strided rows of a
    [P, KD, C] view — v2's only steady-state DMA becomes near-peak."""
    import concourse.mybir as mybir
    import concourse.tile as tile
    from concourse import bacc

    act_fn = getattr(mybir.ActivationFunctionType, act)
    bf16 = mybir.dt.bfloat16
    f32 = mybir.dt.float32

    nc = bacc.Bacc("TRN2", target_bir_lowering=False, debug=False, num_devices=E)
    xT = nc.declare_dram_parameter("xT", [d, C], bf16, isOutput=False)
    w1 = nc.declare_dram_parameter("w1", [d, f], bf16, isOutput=False)
    w2 = nc.declare_dram_parameter("w2", [f, d], bf16, isOutput=False)
    KD = d // P
    KF = f // P
    yT = nc.declare_dram_parameter("yT", [P, KD * C], bf16, isOutput=True)

    tts = []
    off = 0
    while off < C:
        tw = min(tok_tile, C - off)
        tts.append((off, tw))
        off += tw

    from collections import Counter

    tw_counts = Counter(tw for _, tw in tts)
    with tile.TileContext(nc) as tc:
        with (
            tc.tile_pool(name="wpool", bufs=1) as wpool,
            tc.tile_pool(name="xpool", bufs=1) as xpool,
            tc.tile_pool(name="hpool", bufs=KF) as hpool,
            tc.tile_pool(name="ypool", bufs=1) as ypool,
            tc.tile_pool(name="psum", bufs=8, space="PSUM") as psum_pool,
        ):
            w1_sb = wpool.tile([P, KD, f], bf16)
            nc.scalar.dma_start(w1_sb[:], w1.ap().rearrange("(ko p) f -> p ko f", p=P))
            w2_sb = wpool.tile([P, KF, d], bf16)
            nc.scalar.dma_start(w2_sb[:], w2.ap().rearrange("(ko p) f -> p ko f", p=P))

            xT_t = xT.ap().rearrange("(ko p) c -> p ko c", p=P)
            xt_tiles = []
            for t0, tw in tts:
                xt = xpool.tile(
                    [P, KD, tw], bf16, tag=f"xt{tw}", bufs=tw_counts[tw], name="xt"
                )
                nc.scalar.dma_start(xt[:], xT_t[:, :, t0 : t0 + tw])
                xt_tiles.append(xt)

            for (t0, tw), xt in list(zip(tts, xt_tiles)) * repeat:
                h_tiles = []
                for ft in range(KF):
                    ps = psum_pool.tile([P, tok_tile], f32, tag="ps", name="ps")[:, :tw]
                    for k in range(KD):
                        nc.tensor.matmul(
                            ps[:],
                            w1_sb[:, k, ft * P : (ft + 1) * P],
                            xt[:, k, :],
                            start=(k == 0),
                            stop=(k == KD - 1),
                        )
                    h = hpool.tile([P, tok_tile], bf16, tag="h", name="h")[:, :tw]
                    nc.scalar.activation(h[:], ps[:], act_fn)
                    h_tiles.append(h)

                yt = ypool.tile([P, KD * tok_tile], bf16, tag="yt", name="yt")[
                    :, : KD * tw
                ]
                nc.vector.memset(yt[:], 0.0)
                for dt_ in range(KD):
                    ps2 = psum_pool.tile([P, tok_tile], f32, tag="ps", name="ps")[
                        :, :tw
                    ]
                    for ft in range(KF):
                        nc.tensor.matmul(
                            ps2[:],
                            w2_sb[:, ft, dt_ * P : (dt_ + 1) * P],
                            h_tiles[ft][:],
                            start=(ft == 0),
                            stop=(ft == KF - 1),
                        )
                    nc.vector.tensor_copy(yt[:, dt_ * tw : (dt_ + 1) * tw], ps2[:])
                nc.sync.dma_start(yT.ap()[:, KD * t0 : KD * t0 + KD * tw], yt[:])

    nc.compile()
    return nc


def _dispatch_v2c(inputs, tok_tile: int = 512):
    """v2-style expert-per-core dispatch with v2c's slab store layout."""
    x = np.asarray(inputs["x"], dtype=np.float32)
    Wr = np.asarray(inputs["Wr"], dtype=np.float32)
    W1 = np.asarray(inputs["W1"], dtype=np.float32)
    W2 = np.asarray(inputs["W2"], dtype=np.float32)
    B, T, d = x.shape
    N = B * T
    KD = d // P
    xf = np.ascontiguousarray(x.reshape(N, d))

    order, gates2 = _route(xf, Wr)
    counts_arr = np.bincount(order.ravel(), minlength=E)
    C = int(-(-max(int(counts_arr.max()), P) // P) * P)

    idx_list, gate_list = [], []
    for e in range(E):
        tok, slot = np.where(order == e)
        idx_list.append(tok)
        gate_list.append(gates2[tok, slot])

    xf_bf = xf.astype(ml_dtypes.bfloat16)
    in_maps = []
    for e in range(E):
        xTe = np.zeros((d, C), dtype=ml_dtypes.bfloat16)
        tok = idx_list[e]
        xTe[:, : len(tok)] = xf_bf[tok].T
        in_maps.append(
            {
                "xT": xTe,
                "w1": np.ascontiguousarray(W1[e]).astype(ml_dtypes.bfloat16),
                "w2": np.ascontiguousarray(W2[e]).astype(ml_dtypes.bfloat16),
            }
        )

    tts = []
    off = 0
    while off < C:
        tw = min(tok_tile, C - off)
        tts.append((off, tw))
        off += tw

    def combine(results) -> np.ndarray:
        out = np.zeros((N, d), dtype=np.float32)
        for e in range(E):
            y_dev = np.asarray(results[e]["yT"]).astype(np.float32)  # [P, KD*C]
            y_cols = np.empty((d, C), dtype=np.float32)
            for t0, tw in tts:
                blk = y_dev[:, KD * t0 : KD * t0 + KD * tw].reshape(P, KD, tw)
                y_cols[:, t0 : t0 + tw] = blk.transpose(1, 0, 2).reshape(d, tw)
            tok = idx_list[e]
            out[tok] += gate_list[e][:, None] * y_cols[:, : len(tok)].T
        return out.reshape(B, T, d)

    return in_maps, C, combine

